# revision 4
# baseline (speedup 1.0000x reference)
"""LocallyConnected2d Trainium2 kernel (bf16, fully-dense weight stream).

y[b,o,h,w] = sum_{i,ky,kx} x[b,i,h+ky-1,w+kx-1] * weight[i,o,h,w,ky,kx] + bias[o,h,w]

Shapes: x [64,64,32,32], weight [64,64,32,32,3,3], bias [64,32,32] -> y [64,64,32,32].

Strategy
--------
Spatial sharding over H_out: 8 cores x 4 output rows each (x slab with halo).
All compute in bf16, PSUM accumulation in fp32.

The kernel is weight-streaming DMA-bound (weights are used exactly once), so
the packing is 100% dense: zero padded bytes.  Output columns are processed
in PSUM pairs (A=2t, B=2t+1).  Per pair, EIGHT matmuls cover all 18 taps
with no zero weights, using a single x image xb = [x ; shift34(x)] (bottom
half = next padded row, built on-chip by the vector engine):

  J3 [K=64,M=128] x(h+2,2t+1)     : A(2,1), B(2,0)   (stationary on parts 0:64)
  J4 [K=64,M=128] x(h+2,2t+2)     : A(2,2), B(2,1)   (parts 64:128, rhs from
                                                      the shifted bottom half)
  J6 [K=64,M=64]  x(h+2,2t)   -> A(2,0)   } packed in one 64-col block,
  J8 [K=64,M=64]  x(h,  2t+3) -> B(0,2)   } one on each partition half
  J1 [K=128,M=128] [x(h,2t+1);x(h+1,2t+1)] : A(0,1),A(1,1) | B(0,0),B(1,0)
  J5 [K=128,M=64]  [x(h,2t)  ;x(h+1,2t)  ] : A(0,0),A(1,0)
  J7 [K=128,M=64]  [x(h+1,2t+3);x(h+2,2t+3)]: B(1,2),B(2,2)
  J2 [K=128,M=128] [x(h,2t+2);x(h+1,2t+2)] : A(0,2),A(1,2) | B(0,1),B(1,1)

(For h=0 the J6/J8 partition halves swap so each rhs stays in range of its
x-image half.)  Per-pair stationary image = 576 cols x 128 parts, fully
dense = 1152B/partition contiguous runs.  Weight traffic drops 12.58MB ->
9.44MB per core vs. the quadrant-padded packing.

Queues: the sync HWDGE queue streams ONLY the x prefix + weights; x-rest
goes on the scalar queue (concurrent pull), bias on the vector queue.
Output DMAs issue on the scalar queue after their bias adds.  Bias adds
alternate ACT/DVE so neither engine serializes PSUM release.
"""

import sys

sys.path.insert(0, "/opt/trn_rl_repo")

import numpy as np
import ml_dtypes

BF16 = ml_dtypes.bfloat16

B, CIN, COUT, H, W = 64, 64, 64, 32, 32
K = 3
HOUT, WOUT = 32, 32
NCORES = 8
ROWS = HOUT // NCORES  # output rows per core
SLAB_R = ROWS + 2      # x rows needed per core (halo)
SLAB_C = W + 2         # padded width
RC = SLAB_R * SLAB_C   # flattened (row, col) length
NT = WOUT // 2         # column pairs per row
NQ = 2                 # weight chunks per output row
TG = NT // NQ          # pairs per weight chunk (chunk = (h, q))
PCOLS = 576            # stationary image columns per pair (dense)
XSPLIT = 86            # x first-piece: covers all of chunk (0,0)

_nc_cache = {}


def _build_bass():
    import concourse.bass as bass
    import concourse.tile as tile
    from concourse import bacc, mybir

    f32 = mybir.dt.float32
    bf16 = mybir.dt.bfloat16
    nc = bacc.Bacc(None, target_bir_lowering=False)

    x0_d = nc.dram_tensor("x0", (64, RC, B), bf16, kind="ExternalInput")
    wm_d = nc.dram_tensor(
        "wm", (ROWS, NQ, 128, TG, PCOLS), bf16, kind="ExternalInput"
    )
    bias_d = nc.dram_tensor("bias", (128, ROWS, NT), f32, kind="ExternalInput")
    out_d = nc.dram_tensor("out", (ROWS, 128, NT, B), bf16, kind="ExternalOutput")

    with tile.TileContext(nc) as tc:
        with (
            tc.tile_pool(name="xpool", bufs=1) as xpool,
            tc.tile_pool(name="wpool", bufs=4) as wpool,
            tc.tile_pool(name="opool", bufs=2) as opool,
            tc.tile_pool(name="bpool", bufs=1) as bpool,
            tc.tile_pool(name="psum", bufs=8, space=bass.MemorySpace.PSUM) as psum,
        ):
            # xb top half = padded x slab rows 0..5; bottom half = rows
            # shifted by one padded row (+34), built by the vector engine.
            xb = xpool.tile([128, RC, B], bf16, tag="xb")
            nc.sync.dma_start(xb[0:64, 0:XSPLIT, :], x0_d[:, 0:XSPLIT, :])
            # bottom[0:52] <- top[34:86]: unblocks ALL of chunk (0,0)
            nc.vector.tensor_copy(
                xb[64:128, 0 : XSPLIT - 34, :], xb[0:64, 34:XSPLIT, :]
            )

            bi = bpool.tile([128, ROWS, NT], f32, tag="bias")
            nc.scalar.dma_start(bi[:], bias_d[:])

            # x remainder pulls on the scalar queue, concurrent with weights
            nc.scalar.dma_start(xb[0:64, XSPLIT:RC, :], x0_d[:, XSPLIT:RC, :])

            for h in range(ROWS):
                ot = opool.tile([128, NT, B], bf16, tag="out", name="ot")
                for q in range(NQ):
                    wm = wpool.tile([128, TG, PCOLS], bf16, tag="wm", name="wm")
                    if h == 0 and q == 0:
                        # tiny first piece so compute starts sooner
                        nc.sync.dma_start(wm[:, 0:2], wm_d[h, q, :, 0:2])
                        nc.sync.dma_start(wm[:, 2:TG], wm_d[h, q, :, 2:TG])
                    elif h == ROWS - 1 and q == NQ - 1:
                        # small final piece shortens the un-overlapped tail
                        nc.sync.dma_start(wm[:, 0:6], wm_d[h, q, :, 0:6])
                        nc.sync.dma_start(wm[:, 6:TG], wm_d[h, q, :, 6:TG])
                    else:
                        nc.sync.dma_start(wm[:], wm_d[h, q])

                    if h == 0 and q == 0:
                        # rest of the shifted bottom half (DVE), after x-rest
                        nc.vector.tensor_copy(
                            xb[64:128, XSPLIT - 34 : 111, :],
                            xb[0:64, XSPLIT : 111 + 34, :],
                        )
                        nc.vector.tensor_copy(
                            xb[64:128, 111 : RC - 34, :], xb[0:64, 111 + 34 : RC, :]
                        )

                    for tt in range(TG):
                        t = q * TG + tt
                        ps = psum.tile([128, B], f32, tag="ps")
                        mm = nc.tensor.matmul
                        # PE quadrant matmuls (K=64) at different row
                        # positions run CONCURRENTLY when their tile regions
                        # are disjoint; two quadrants writing overlapping
                        # PSUM ranges fault on HW.  Full-K matmuls occupy
                        # the whole array and act as serializers, so the
                        # order below keeps every concurrent pair
                        # PSUM-disjoint: J3, J1(full), J4, J2(full),
                        # J6/J8 (disjoint halves), J5, J7 (disjoint).
                        # J3: x(h+2,2t+1) -> A(2,1) | B(2,0)
                        mm(
                            ps[:],
                            wm[0:64, tt, 384:512],
                            xb[0:64, (h + 2) * SLAB_C + 2 * t + 1, :],
                            start=True,
                            stop=False,
                        )
                        # J1: [x(h,2t+1);x(h+1,2t+1)]
                        mm(
                            ps[:],
                            wm[:, tt, 0:128],
                            xb[:, h * SLAB_C + 2 * t + 1, :],
                            start=False,
                            stop=False,
                        )
                        # J4: x(h+2,2t+2) via bottom half -> A(2,2) | B(2,1)
                        mm(
                            ps[:],
                            wm[64:128, tt, 384:512],
                            xb[64:128, (h + 1) * SLAB_C + 2 * t + 2, :],
                            start=False,
                            stop=False,
                        )
                        # J2: [x(h,2t+2);x(h+1,2t+2)]
                        mm(
                            ps[:],
                            wm[:, tt, 128:256],
                            xb[:, h * SLAB_C + 2 * t + 2, :],
                            start=False,
                            stop=False,
                        )
                        if h == 0:
                            # J8 on top parts: x(0,2t+3) -> B(0,2)
                            mm(
                                ps[64:128, :],
                                wm[0:64, tt, 512:576],
                                xb[0:64, 2 * t + 3, :],
                                start=False,
                                stop=False,
                            )
                            # J6 on bottom parts: x(2,2t) -> A(2,0)
                            mm(
                                ps[0:64, :],
                                wm[64:128, tt, 512:576],
                                xb[64:128, SLAB_C + 2 * t, :],
                                start=False,
                                stop=False,
                            )
                        else:
                            # J6: x(h+2,2t) -> A(2,0)
                            mm(
                                ps[0:64, :],
                                wm[0:64, tt, 512:576],
                                xb[0:64, (h + 2) * SLAB_C + 2 * t, :],
                                start=False,
                                stop=False,
                            )
                            # J8: x(h,2t+3) via bottom half -> B(0,2)
                            mm(
                                ps[64:128, :],
                                wm[64:128, tt, 512:576],
                                xb[64:128, (h - 1) * SLAB_C + 2 * t + 3, :],
                                start=False,
                                stop=False,
                            )
                        # J5: [x(h,2t);x(h+1,2t)] -> A only
                        mm(
                            ps[0:64, :],
                            wm[:, tt, 256:320],
                            xb[:, h * SLAB_C + 2 * t, :],
                            start=False,
                            stop=False,
                        )
                        # J7: [x(h+1,2t+3);x(h+2,2t+3)] -> B only
                        mm(
                            ps[64:128, :],
                            wm[:, tt, 320:384],
                            xb[:, (h + 1) * SLAB_C + 2 * t + 3, :],
                            start=False,
                            stop=True,
                        )
                        if t % 2 == 0:
                            nc.scalar.activation(
                                ot[:, t, :],
                                ps[:],
                                mybir.ActivationFunctionType.Identity,
                                bias=bi[:, h, t : t + 1],
                            )
                        else:
                            nc.vector.tensor_scalar_add(
                                ot[:, t, :], ps[:], bi[:, h, t : t + 1]
                            )

                    # output on the scalar HWDGE queue (never stalls the
                    # weight stream); final piece split to shorten the tail
                    if h == ROWS - 1 and q == NQ - 1:
                        nc.scalar.dma_start(
                            out_d[h, :, q * TG : q * TG + 6, :],
                            ot[:, q * TG : q * TG + 6, :],
                        )
                        nc.scalar.dma_start(
                            out_d[h, :, q * TG + 6 : NT, :],
                            ot[:, q * TG + 6 : NT, :],
                        )
                    else:
                        nc.scalar.dma_start(
                            out_d[h, :, q * TG : (q + 1) * TG, :],
                            ot[:, q * TG : (q + 1) * TG, :],
                        )

    nc.compile()
    return nc


def get_nc():
    if "nc" not in _nc_cache:
        _nc_cache["nc"] = _build_bass()
    return _nc_cache["nc"]


def pack_inputs(x, weight, bias):
    """Returns list of per-core in_maps (numpy, C-contiguous)."""
    x = np.asarray(x, dtype=np.float32)
    weight = np.asarray(weight, dtype=np.float32)
    bias = np.asarray(bias, dtype=np.float32)

    # padded x: [B, CIN, H+2, W+2]
    xp = np.zeros((B, CIN, H + 2, W + 2), dtype=np.float32)
    xp[:, :, 1:-1, 1:-1] = x

    # weight -> [h, w, ky, kx, cin, cout] in bf16
    wt_all = np.ascontiguousarray(
        np.transpose(weight, (2, 3, 4, 5, 0, 1))
    ).astype(BF16)

    in_maps = []
    for c in range(NCORES):
        h0 = c * ROWS
        # x slab rows h0-1 .. h0+ROWS (SLAB_R rows of padded x)
        slab = xp[:, :, h0 : h0 + SLAB_R, :]  # [B, CIN, SLAB_R, SLAB_C]
        x0 = np.transpose(slab, (1, 2, 3, 0)).reshape(CIN, RC, B).astype(BF16)

        wh = wt_all[h0 : h0 + ROWS]  # [ROWS, w, ky, kx, cin, cout]
        A = wh[:, 0::2]  # [ROWS, NT, ky, kx, cin, cout]
        Bw = wh[:, 1::2]

        # dense stationary image [ROWS, 128, NT, 576]
        wimg = np.empty((ROWS, 128, NT, PCOLS), dtype=BF16)
        # J1 [0:128]: A(0,1),A(1,1) | B(0,0),B(1,0)
        wimg[:, 0:64, :, 0:64] = A[:, :, 0, 1].transpose(0, 2, 1, 3)
        wimg[:, 64:128, :, 0:64] = A[:, :, 1, 1].transpose(0, 2, 1, 3)
        wimg[:, 0:64, :, 64:128] = Bw[:, :, 0, 0].transpose(0, 2, 1, 3)
        wimg[:, 64:128, :, 64:128] = Bw[:, :, 1, 0].transpose(0, 2, 1, 3)
        # J2 [128:256]: A(0,2),A(1,2) | B(0,1),B(1,1)
        wimg[:, 0:64, :, 128:192] = A[:, :, 0, 2].transpose(0, 2, 1, 3)
        wimg[:, 64:128, :, 128:192] = A[:, :, 1, 2].transpose(0, 2, 1, 3)
        wimg[:, 0:64, :, 192:256] = Bw[:, :, 0, 1].transpose(0, 2, 1, 3)
        wimg[:, 64:128, :, 192:256] = Bw[:, :, 1, 1].transpose(0, 2, 1, 3)
        # J5 [256:320]: A(0,0);A(1,0)
        wimg[:, 0:64, :, 256:320] = A[:, :, 0, 0].transpose(0, 2, 1, 3)
        wimg[:, 64:128, :, 256:320] = A[:, :, 1, 0].transpose(0, 2, 1, 3)
        # J7 [320:384]: B(1,2);B(2,2)
        wimg[:, 0:64, :, 320:384] = Bw[:, :, 1, 2].transpose(0, 2, 1, 3)
        wimg[:, 64:128, :, 320:384] = Bw[:, :, 2, 2].transpose(0, 2, 1, 3)
        # J3 (parts 0:64) [384:512]: A(2,1) | B(2,0)
        wimg[:, 0:64, :, 384:448] = A[:, :, 2, 1].transpose(0, 2, 1, 3)
        wimg[:, 0:64, :, 448:512] = Bw[:, :, 2, 0].transpose(0, 2, 1, 3)
        # J4 (parts 64:128) [384:512]: A(2,2) | B(2,1)
        wimg[:, 64:128, :, 384:448] = A[:, :, 2, 2].transpose(0, 2, 1, 3)
        wimg[:, 64:128, :, 448:512] = Bw[:, :, 2, 1].transpose(0, 2, 1, 3)
        # J6/J8 [512:576]: h>=1: J6=A(2,0) top / J8=B(0,2) bottom; h=0 swapped
        wimg[1:, 0:64, :, 512:576] = A[1:, :, 2, 0].transpose(0, 2, 1, 3)
        wimg[1:, 64:128, :, 512:576] = Bw[1:, :, 0, 2].transpose(0, 2, 1, 3)
        wimg[0, 0:64, :, 512:576] = Bw[0, :, 0, 2].transpose(1, 0, 2)
        wimg[0, 64:128, :, 512:576] = A[0, :, 2, 0].transpose(1, 0, 2)

        # [ROWS, 128, NT, 576] -> [ROWS, NQ, 128, TG, 576]
        wm = wimg.reshape(ROWS, 128, NQ, TG, PCOLS).transpose(0, 2, 1, 3, 4)

        # bias image [128, ROWS, NT]: partition s*64+o -> (w=2t+s, cout=o)
        bh = bias[:, h0 : h0 + ROWS, :]  # [cout, ROWS, W]
        bimg = np.concatenate([bh[:, :, 0::2], bh[:, :, 1::2]], axis=0)

        in_maps.append(
            {
                "x0": np.ascontiguousarray(x0),
                "wm": np.ascontiguousarray(wm),
                "bias": np.ascontiguousarray(bimg),
            }
        )
    return in_maps


def unpack_outputs(results):
    """results: list of per-core out_maps with 'out' [ROWS, 128, NT, B] bf16."""
    full = np.concatenate([np.asarray(r["out"]) for r in results], axis=0)
    # [HOUT, 2, COUT, NT, B] -> [B, COUT, HOUT, NT, 2]
    o = full.reshape(HOUT, 2, COUT, NT, B)
    y = np.transpose(o, (4, 2, 0, 3, 1)).reshape(B, COUT, HOUT, WOUT)
    return np.ascontiguousarray(y.astype(np.float32))


def run(in_maps, **kwargs):
    from concourse import bass_utils

    nc = get_nc()
    return bass_utils.run_bass_kernel_spmd(
        nc, in_maps, core_ids=list(range(NCORES)), **kwargs
    )


def kernel(x, weight, bias):
    in_maps = pack_inputs(x, weight, bias)
    res = run(in_maps)
    return unpack_outputs(res.results)


if __name__ == "__main__":
    rng = np.random.default_rng(0)
    x = rng.standard_normal((B, CIN, H, W), dtype=np.float32)
    weight = rng.standard_normal((CIN, COUT, HOUT, WOUT, K, K), dtype=np.float32)
    bias = rng.standard_normal((COUT, HOUT, WOUT), dtype=np.float32)
    y = kernel(x, weight, bias)
    print("out", y.shape, y.dtype)
